# revision 52
# baseline (speedup 1.0000x reference)
"""Trainium2 Bass kernel for BasicMambaBlock (B=2, L=2048, d_model=1024).

Sharding: 8 cores = 2 batch groups x 4 TP shards.
Mamba half: d_inner TP-4 (512 ch/core), feature-major activations,
scan via tensor_tensor_scan in two L-halves with chained state; the
y = sum_n C_n*h_n accumulation runs on the PE (identity-matmul into PSUM).
out_proj partials ReduceScatter'd so each core owns an L/4 token slice.
FF half: token-sliced (512 tok/core) with FULL GEGLU weights streamed in
4 column chunks; residual added on-chip, single (512,1024) fp32 output.
All activations except the FF gelu are composed from Exp/Ln only (silu via
sigmoid(v) = exp(-ln(1+exp(-v)))) and the act-table chooser is steered to
the combined natural_log_exp set, so the scalar engine loads an activation
table ~3 times total instead of ~54.
"""
import sys

sys.path.insert(0, "/opt/trn_rl_repo")

import numpy as np
import ml_dtypes
from contextlib import ExitStack

import concourse.bass as bass
import concourse.tile as tile
from concourse import bacc, mybir
from concourse.bass_utils import run_bass_kernel_spmd

FP32 = mybir.dt.float32
BF16 = mybir.dt.bfloat16
ALU = mybir.AluOpType
ACTF = mybir.ActivationFunctionType
NPBF16 = ml_dtypes.bfloat16

import os

# Timing-only ablation: replace the two collectives with local DMA copies
# (results become WRONG; used to isolate HW collective cost).
NOCOLL = bool(int(os.environ.get("KB_NOCOLL", "0")))

DM = 1024          # d_model
DI = 2048          # d_inner (global)
DIS = DI // 4      # 512 per-core d_inner shard
NST = 16           # d_state
RNK = 64           # dt_rank
DC = 4             # conv width
FFI = 4096         # ff inner (global)
EPS = 1e-5
L_FULL = 2048
B_FULL = 2
LS = L_FULL // 4   # 512-token output slice per core


def _silu_div(nc, scr, out, x_ap, d_scr_tag, bias=None, nbias=None):
    """out = silu(x + b) = (x + b) * sigmoid(x + b), with
    sigmoid(v) = exp(-softplus(-v)) = exp(-ln(1 + exp(-v))).

    3 Act ops (Exp/Ln/Exp, all in the natural_log_exp table set used by the
    rest of the kernel) plus 1 DVE op. bias/nbias are (128,1) APs for +b
    and -b (None -> 0).
    """
    shape = [x_ap.shape[0], x_ap.free_size()]
    e = scr.tile(shape, BF16, tag=d_scr_tag + "_e")
    nc.scalar.activation(e[:], x_ap, ACTF.Exp, scale=-1.0,
                         bias=(nbias if nbias is not None else 0.0))
    l = scr.tile(shape, BF16, tag=d_scr_tag + "_l")
    nc.scalar.activation(l[:], e[:], ACTF.Ln, bias=1.0)
    s = scr.tile(shape, BF16, tag=d_scr_tag + "_s")
    nc.scalar.activation(s[:], l[:], ACTF.Exp, scale=-1.0)
    if bias is None:
        nc.vector.scalar_tensor_tensor(out, x_ap, 1.0, s[:],
                                       ALU.mult, ALU.mult)
    else:
        nc.vector.scalar_tensor_tensor(out, x_ap, bias, s[:],
                                       ALU.add, ALU.mult)


def _layer_norm_stage(nc, tc, ctx, src_tiles, n_tok_tiles, ident_sb, g_ap, b_ap,
                      hfm_pool, L, name):
    """Token-major LN on src_tiles (list of (128, DM) fp32 sbuf tiles) ->
    feature-major bf16 tiles (8 x (128, L)), with g/b applied per-partition
    after the transpose. Returns list of 8 hfm tiles."""
    stat = ctx.enter_context(tc.tile_pool(name=f"{name}_stat", bufs=4))
    scr = ctx.enter_context(tc.tile_pool(name=f"{name}_scr", bufs=2))
    nrm = ctx.enter_context(tc.tile_pool(name=f"{name}_nrm", bufs=6))
    gsb = ctx.enter_context(tc.tile_pool(name=f"{name}_gb", bufs=1))

    # g/b per-feature: 8 x (128,1) tiles
    g_t, b_t = [], []
    for f in range(DM // 128):
        t = gsb.tile([128, 1], FP32, tag=f"g{f}")
        nc.sync.dma_start(out=t[:], in_=g_ap[f * 128:(f + 1) * 128, :])
        g_t.append(t)
        t = gsb.tile([128, 1], FP32, tag=f"b{f}")
        nc.sync.dma_start(out=t[:], in_=b_ap[f * 128:(f + 1) * 128, :])
        b_t.append(t)

    eps_t = gsb.tile([128, 1], FP32, tag="eps")
    nc.vector.memset(eps_t[:], EPS)

    # Group-of-4 processing: stats packed into columns, batched mu/rstd
    # math on (128,4) slices, then normalize + transpose the group so the
    # first feature-major chunks are ready before the full LN finishes.
    nt_ = n_tok_tiles
    TCH = 4  # token tiles per group / transpose chunk
    s1p = stat.tile([128, nt_], FP32, tag="s1p")
    s2p = stat.tile([128, nt_], FP32, tag="s2p")
    mu = stat.tile([128, nt_], FP32, tag="mu")
    musq = stat.tile([128, nt_], FP32, tag="musq")
    var = stat.tile([128, nt_], FP32, tag="var")
    lv = stat.tile([128, nt_], FP32, tag="lv")
    rstd = stat.tile([128, nt_], FP32, tag="rstd")
    psT = ctx.enter_context(tc.tile_pool(name=f"{name}_psT", bufs=2,
                                         space="PSUM"))
    hfm = []
    for f in range(DM // 128):
        ht = hfm_pool.tile([128, L], BF16, tag="hfm")
        hfm.append(ht)

    for c in range(n_tok_tiles // TCH):
        g0 = c * TCH
        gs = slice(g0, g0 + TCH)
        for i in range(g0, g0 + TCH):
            xt = src_tiles[i]
            nc.vector.tensor_reduce(s1p[:, i:i + 1], xt[:],
                                    mybir.AxisListType.X, ALU.add)
            sq = scr.tile([128, DM], FP32, tag="sq")
            nc.scalar.activation(sq[:], xt[:], ACTF.Square,
                                 accum_out=s2p[:, i:i + 1])
        nc.vector.tensor_scalar_mul(mu[:, gs], s1p[:, gs], 1.0 / DM)
        nc.vector.tensor_mul(musq[:, gs], mu[:, gs], mu[:, gs])
        nc.vector.scalar_tensor_tensor(var[:, gs], s2p[:, gs], 1.0 / DM,
                                       musq[:, gs], ALU.mult, ALU.subtract)
        nc.scalar.activation(lv[:, gs], var[:, gs], ACTF.Ln, bias=eps_t[:])
        nc.scalar.activation(rstd[:, gs], lv[:, gs], ACTF.Exp, scale=-0.5)
        normed = []
        for i in range(g0, g0 + TCH):
            nt = nrm.tile([128, DM], BF16, tag="normed")
            nc.vector.tensor_scalar(nt[:], src_tiles[i][:], mu[:, i:i + 1],
                                    rstd[:, i:i + 1], ALU.subtract, ALU.mult)
            normed.append(nt)
        for f in range(DM // 128):
            pt = psT.tile([128, TCH * 128], BF16, tag="psT")
            for i in range(TCH):
                nc.tensor.transpose(pt[:, i * 128:(i + 1) * 128],
                                    normed[i][:, f * 128:(f + 1) * 128],
                                    ident_sb[:])
            nc.any.tensor_scalar(
                hfm[f][:, g0 * 128:(g0 + TCH) * 128], pt[:],
                g_t[f][:], b_t[f][:], ALU.mult, ALU.add)
    return hfm


def build_nc(L=L_FULL):
    n_tok = L // 128
    CH = min(512, L)
    n_ch = L // CH  # token chunks for matmul moving dim
    LH = L // 2     # scan half length
    n_res = LS // 128  # token tiles in this core's residual/FF slice

    nc = bacc.Bacc("TRN2", target_bir_lowering=False, debug=False,
                   num_devices=8)

    # The act-table chooser is first-match per function, so an Exp/Ln mix
    # alternates between the exp-only and ln-only sets (a ~1.3us table load
    # per switch). Steer both functions to the combined natural_log_exp set
    # (a real act_info set containing exp+ln) by hiding them in the others.
    from concourse.hw_specs import get_activation_tables
    try:
        tabs = get_activation_tables(nc.m.arch)
        for tname, tset in tabs.items():
            if tname != "natural_log_exp_and_others":
                tset.discard(ACTF.Exp)
                tset.discard(ACTF.Ln)
    except Exception:
        pass

    # ---- dram params ----
    def din(name, shape, dt=FP32):
        return nc.dram_tensor(name, shape, dt, kind="ExternalInput").ap()

    x_d = din("x", [L, DM], BF16)
    x_res_d = din("x_res", [LS, DM])               # this core's token slice
    ln1_g = din("ln1_g", [DM, 1]); ln1_b = din("ln1_b", [DM, 1])
    ln2_g = din("ln2_g", [DM, 1]); ln2_b = din("ln2_b", [DM, 1])
    w_in_d = din("w_in", [DM, 2 * DIS], BF16)      # [xc cols | z cols]
    conv_w_d = din("conv_w", [DIS, DC])
    conv_b_d = din("conv_b", [DIS, 1])
    a_neg_d = din("a_neg", [DIS, NST])             # A = -exp(a_log) shard
    w_x_d = din("w_x", [DIS, RNK + 2 * NST], BF16)
    w_dt_d = din("w_dt", [RNK, DIS], BF16)
    b_dt_d = din("b_dt", [DIS, 1])
    d_skip_d = din("d_skip", [DIS, 1])
    w_out_d = din("w_out", [DIS, DM], BF16)
    w_ff1_d = din("w_ff1", [DM, 2 * FFI], BF16)    # FULL [a 4096 | g 4096]
    b_ff1_d = din("b_ff1", [2 * FFI, 1])           # FULL
    w_ff2_d = din("w_ff2", [FFI, DM], BF16)        # FULL
    ident_d = din("ident", [128, 128], BF16)

    out_d = nc.dram_tensor("out", [LS, DM], FP32, kind="ExternalOutput").ap()

    NCHK = 4               # FF column chunks
    FCH = FFI // NCHK      # 1024 a-cols + 1024 g-cols per chunk

    with tile.TileContext(nc) as tc, ExitStack() as octx:
        dram = octx.enter_context(tc.tile_pool(name="dram", bufs=1,
                                               space="DRAM"))
        const = octx.enter_context(tc.tile_pool(name="const", bufs=1))

        ident_sb = const.tile([128, 128], BF16, tag="ident")
        nc.sync.dma_start(out=ident_sb[:], in_=ident_d[:, :])

        # dram intermediates for collectives
        dbc_part = dram.tile([RNK + 2 * NST, L], BF16, tag="dbc_part")
        dbc_ar = dram.tile([RNK + 2 * NST, L], BF16, tag="dbc_ar")
        m_part = dram.tile([L, DM], BF16, tag="m_part")
        m_rs = dram.tile([LS, DM], BF16, tag="m_rs")

        groups = [[0, 1, 2, 3], [4, 5, 6, 7]]

        # Long-lived pools (phases 2-4) created first so shorter-lived
        # pools can pop in LIFO order before phase 5 reuses the space.
        pMain = octx.enter_context(ExitStack())
        wts = pMain.enter_context(tc.tile_pool(name="wts", bufs=1))
        sconst = pMain.enter_context(tc.tile_pool(name="sconst", bufs=1))
        act = pMain.enter_context(tc.tile_pool(name="act", bufs=1))
        zsp = pMain.enter_context(tc.tile_pool(name="zs", bufs=1))
        dtp = pMain.enter_context(tc.tile_pool(name="dtp", bufs=1))
        dtxp = pMain.enter_context(tc.tile_pool(name="dtx", bufs=1))
        dbcp = pMain.enter_context(tc.tile_pool(name="dbcp", bufs=1))

        # ================= Phase 1: LN1 -> h_fm =================
        p12 = pMain.enter_context(ExitStack())
        mm = p12.enter_context(tc.tile_pool(name="mm", bufs=4, space="PSUM"))
        hfm_pool = p12.enter_context(tc.tile_pool(name="hfm", bufs=8))
        with ExitStack() as p1, nc.named_scope("p1_ln1"):
            xload = p1.enter_context(tc.tile_pool(name="xload", bufs=6))
            xt_list = []
            for i in range(n_tok):
                xt = xload.tile([128, DM], BF16, tag="xt")
                nc.sync.dma_start(out=xt[:], in_=x_d[i * 128:(i + 1) * 128, :])
                xt_list.append(xt)
            hfm = _layer_norm_stage(nc, tc, p1, xt_list, n_tok, ident_sb,
                                    ln1_g, ln1_b, hfm_pool, L, "ln1")

        # ================= Phase 2: in_proj, conv, dbc, dt =================
        _sid2 = nc.enter_named_scope("p2_inproj", False)[0]
        w12 = p12.enter_context(tc.tile_pool(name="w12", bufs=1))
        w_in_sb = []
        for k in range(8):
            t = w12.tile([128, 2 * DIS], BF16, tag=f"w_in{k}")
            nc.sync.dma_start(out=t[:], in_=w_in_d[k * 128:(k + 1) * 128, :])
            w_in_sb.append(t)
        wx_sb = []
        for k in range(4):
            t = w12.tile([128, RNK + 2 * NST], BF16, tag=f"wx{k}")
            nc.sync.dma_start(out=t[:], in_=w_x_d[k * 128:(k + 1) * 128, :])
            wx_sb.append(t)
        wdt_sb = w12.tile([RNK, DIS], BF16, tag="wdt")
        nc.sync.dma_start(out=wdt_sb[:], in_=w_dt_d[:, :])

        cw_sb, cb_sb, cbn_sb, a_sb, bdt_sb, dskip_sb = [], [], [], [], [], []
        for d in range(4):
            r = slice(d * 128, (d + 1) * 128)
            t = sconst.tile([128, DC], FP32, tag=f"cw{d}")
            nc.sync.dma_start(out=t[:], in_=conv_w_d[r, :]); cw_sb.append(t)
            t = sconst.tile([128, 1], FP32, tag=f"cb{d}")
            nc.sync.dma_start(out=t[:], in_=conv_b_d[r, :]); cb_sb.append(t)
            tn = sconst.tile([128, 1], FP32, tag=f"cbn{d}")
            nc.vector.tensor_scalar_mul(tn[:], t[:], -1.0); cbn_sb.append(tn)
            t = sconst.tile([128, NST], FP32, tag=f"a{d}")
            nc.sync.dma_start(out=t[:], in_=a_neg_d[r, :]); a_sb.append(t)
            t = sconst.tile([128, 1], FP32, tag=f"bdt{d}")
            nc.sync.dma_start(out=t[:], in_=b_dt_d[r, :]); bdt_sb.append(t)
            t = sconst.tile([128, 1], FP32, tag=f"dsk{d}")
            nc.sync.dma_start(out=t[:], in_=d_skip_d[r, :]); dskip_sb.append(t)

        xc_pad = []
        for d in range(4):
            t = act.tile([128, L + 3], BF16, tag=f"xcp{d}")
            nc.vector.memset(t[:, 0:3], 0.0)
            xc_pad.append(t)
        zraw = p12.enter_context(tc.tile_pool(name="zraw", bufs=1))
        z_sb = [zraw.tile([128, L], BF16, tag=f"z{d}", name=f"z{d}")
                for d in range(4)]

        # in_proj xc columns only (z columns are deferred into the
        # dbc-AllReduce window below)
        def inproj_f(f):
            for c in range(n_ch):
                ps = mm.tile([128, CH], FP32, tag="mm", name="ps")
                for k in range(8):
                    nc.tensor.matmul(
                        ps[:], w_in_sb[k][:, f * 128:(f + 1) * 128],
                        hfm[k][:, c * CH:(c + 1) * CH],
                        start=(k == 0), stop=(k == 7))
                if f < 4:
                    nc.any.tensor_copy(
                        xc_pad[f][:, 3 + c * CH: 3 + (c + 1) * CH], ps[:])
                else:
                    nc.any.tensor_copy(
                        z_sb[f - 4][:, c * CH:(c + 1) * CH], ps[:])

        for f in range(4):
            inproj_f(f)

        # conv + silu (writes silu'd xc back into xc_pad[:, 3:3+L])
        cacc = p12.enter_context(tc.tile_pool(name="cacc", bufs=2))
        for d in range(4):
            acc = cacc.tile([128, L], BF16, tag="cacc")
            nc.vector.tensor_scalar_mul(acc[:], xc_pad[d][:, 0:L],
                                        cw_sb[d][:, 0:1])
            for j in range(1, DC):
                nc.vector.scalar_tensor_tensor(
                    acc[:], xc_pad[d][:, j:j + L], cw_sb[d][:, j:j + 1],
                    acc[:], ALU.mult, ALU.add)
            _silu_div(nc, cacc, xc_pad[d][:, 3:3 + L], acc[:], "sil",
                      bias=cb_sb[d][:], nbias=cbn_sb[d][:])

        # dbc partial + AllReduce
        nc.leave_named_scope("p2_inproj", _sid2, False)
        _sid2b = nc.enter_named_scope("p2b_dbc_ar", False)[0]
        dbc_sb = dbcp.tile([RNK + 2 * NST, L], BF16, tag="dbc")
        for c in range(n_ch):
            ps = mm.tile([RNK + 2 * NST, CH], FP32, tag="mm")
            for k in range(4):
                nc.tensor.matmul(ps[:], wx_sb[k][:],
                                 xc_pad[k][:, 3 + c * CH:3 + (c + 1) * CH],
                                 start=(k == 0), stop=(k == 3))
            nc.any.tensor_copy(dbc_sb[:, c * CH:(c + 1) * CH], ps[:])
        nc.gpsimd.dma_start(out=dbc_part[:], in_=dbc_sb[:])
        if NOCOLL:
            nc.sync.dma_start(out=dbc_ar[:], in_=dbc_part[:])
        else:
            nc.gpsimd.collective_compute(
                "AllReduce", ALU.add, replica_groups=groups,
                ins=[dbc_part.opt()], outs=[dbc_ar.opt()])

        # ---- overlapped with the AllReduce: z in_proj, z-silu, prefetches
        for f in range(4, 8):
            inproj_f(f)
        zs_sb = []
        for d in range(4):
            zs = zsp.tile([128, L], BF16, tag=f"zs{d}")
            _silu_div(nc, cacc, zs[:], z_sb[d][:], "sil")
            zs_sb.append(zs)
        wout_sb = []
        for k in range(4):
            t = wts.tile([128, DM], BF16, tag=f"wout{k}")
            nc.sync.dma_start(out=t[:], in_=w_out_d[k * 128:(k + 1) * 128, :])
            wout_sb.append(t)

        nc.sync.dma_start(out=dbc_sb[:], in_=dbc_ar[:])
        dtlo = dbc_sb[0:RNK, :]
        nc.leave_named_scope("p2b_dbc_ar", _sid2b, False)
        _sid2c = nc.enter_named_scope("p2c_dt", False)[0]

        # dt = softplus(dt_lo @ w_dt + b_dt) = Ln(Exp(u + b_dt) + 1)
        # chunk-outer so the scan's half-0 inputs (dt/dtx chunks 0,1 of
        # every d) are ready as early as possible
        spscr = p12.enter_context(tc.tile_pool(name="spscr", bufs=3))
        dt_sb = [dtp.tile([128, L], BF16, tag=f"dt{d}", name=f"dt{d}")
                 for d in range(4)]
        dtx_sb = [dtxp.tile([128, L], BF16, tag=f"dtx{d}", name=f"dtx{d}")
                  for d in range(4)]
        for c in range(n_ch):
            cs = slice(c * CH, (c + 1) * CH)
            for d in range(4):
                ps = mm.tile([128, CH], FP32, tag="mm", name="ps")
                nc.tensor.matmul(ps[:], wdt_sb[:, d * 128:(d + 1) * 128],
                                 dtlo[:, cs].opt(),
                                 start=True, stop=True)
                e = spscr.tile([128, CH], FP32, tag="sp_e")
                nc.scalar.activation(e[:], ps[:], ACTF.Exp, bias=bdt_sb[d][:])
                nc.scalar.activation(dt_sb[d][:, cs], e[:], ACTF.Ln, bias=1.0)
                nc.vector.tensor_mul(dtx_sb[d][:, cs], dt_sb[d][:, cs],
                                     xc_pad[d][:, 3 + c * CH:3 + (c + 1) * CH])
        p12.close()

        nc.leave_named_scope("p2c_dt", _sid2c, False)
        _sid3 = nc.enter_named_scope("p3_scan", False)[0]

        # ================= Phase 3: scan + gate =================
        # Two L-halves; y_d accumulated in PSUM via identity matmul over n.
        p34 = pMain.enter_context(ExitStack())
        gp = p34.enter_context(tc.tile_pool(name="gated", bufs=1))
        p3 = p34.enter_context(ExitStack())
        sc = p3.enter_context(tc.tile_pool(name="scan", bufs=4))
        bcp = p3.enter_context(tc.tile_pool(name="bcast", bufs=5))
        ypsum = p3.enter_context(tc.tile_pool(name="ypsum", bufs=1,
                                              space="PSUM"))
        hlast = [sconst.tile([128, NST], FP32, tag=f"hl{d}", name=f"hl{d}")
                 for d in range(4)]
        gated_sb = [gp.tile([128, L], BF16, tag=f"g{d}", name=f"g{d}")
                    for d in range(4)]

        for h in range(2):
            off = h * LH
            y_ps = [ypsum.tile([128, LH], FP32, tag=f"yps{d}", name=f"yps{d}")
                    for d in range(4)]
            for n in range(NST):
                bcB = bcp.tile([128, LH], BF16, tag="bcB")
                nc.sync.dma_start(
                    out=bcB[:],
                    in_=dbc_ar[RNK + n:RNK + n + 1,
                               off:off + LH].broadcast_to([128, LH]))
                bcC = bcp.tile([128, LH], BF16, tag="bcC")
                nc.sync.dma_start(
                    out=bcC[:],
                    in_=dbc_ar[RNK + NST + n:RNK + NST + n + 1,
                               off:off + LH].broadcast_to([128, LH]))
                for d in range(4):
                    dA = sc.tile([128, LH], BF16, tag="dA")
                    nc.scalar.activation(dA[:], dt_sb[d][:, off:off + LH],
                                         ACTF.Exp, scale=a_sb[d][:, n:n + 1])
                    dB = sc.tile([128, LH], BF16, tag="dB")
                    nc.vector.tensor_mul(dB[:], dtx_sb[d][:, off:off + LH],
                                         bcB[:])
                    # h state scan (in-place over dB), fp32 internal state
                    init = 0.0 if h == 0 else hlast[d][:, n:n + 1]
                    nc.vector.tensor_tensor_scan(dB[:], dA[:], dB[:], init,
                                                 ALU.mult, ALU.add)
                    if h == 0:
                        nc.scalar.activation(hlast[d][:, n:n + 1],
                                             dB[:, LH - 1:LH], ACTF.Identity)
                    # hC into dA tile (reuse), then accumulate on PE
                    # (matmul output is capped at one PSUM bank = 512 fp32).
                    # Half the hC muls go to the otherwise-idle Pool engine
                    # to offload the DVE (the scan-phase bottleneck).
                    if d < 3:
                        nc.gpsimd.tensor_mul(dA[:], dB[:], bcC[:])
                    else:
                        nc.vector.tensor_mul(dA[:], dB[:], bcC[:])
                    for q in range(LH // 512):
                        nc.tensor.matmul(
                            y_ps[d][:, q * 512:(q + 1) * 512], ident_sb[:],
                            dA[:, q * 512:(q + 1) * 512],
                            start=(n == 0), stop=(n == NST - 1))
            # gate: gated = (y + d_skip*xc) * silu(z)
            for d in range(4):
                tmp = sc.tile([128, LH], FP32, tag="gtmp")
                nc.vector.scalar_tensor_tensor(
                    tmp[:], xc_pad[d][:, 3 + off:3 + off + LH],
                    dskip_sb[d][:], y_ps[d][:], ALU.mult, ALU.add)
                nc.gpsimd.tensor_mul(gated_sb[d][:, off:off + LH], tmp[:],
                                      zs_sb[d][:, off:off + LH])

        nc.leave_named_scope("p3_scan", _sid3, False)
        _sid4 = nc.enter_named_scope("p4_outproj", False)[0]

        # ================= Phase 4: out_proj + ReduceScatter =================
        p3.close()
        mmo = p34.enter_context(tc.tile_pool(name="mmo", bufs=2, space="PSUM"))
        mm2 = p34.enter_context(tc.tile_pool(name="mm2", bufs=2, space="PSUM"))
        mp_pool = p34.enter_context(tc.tile_pool(name="mp", bufs=3))
        for i in range(n_tok):
            mp = mp_pool.tile([128, DM], BF16, tag="mp")
            for nchk in range(2):
                pool = mmo if nchk == 0 else mm2
                ps = pool.tile([128, 512], FP32, tag=pool.name)
                for k in range(4):
                    nc.tensor.matmul(
                        ps[:], gated_sb[k][:, i * 128:(i + 1) * 128],
                        wout_sb[k][:, nchk * 512:(nchk + 1) * 512],
                        start=(k == 0), stop=(k == 3))
                nc.any.tensor_copy(mp[:, nchk * 512:(nchk + 1) * 512], ps[:])
            nc.sync.dma_start(out=m_part[i * 128:(i + 1) * 128, :], in_=mp[:])
        if NOCOLL:
            nc.sync.dma_start(out=m_rs[:], in_=m_part[0:LS, :])
        else:
            nc.gpsimd.collective_compute(
                "ReduceScatter", ALU.add, replica_groups=groups,
                ins=[m_part.opt()], outs=[m_rs.opt()])
        p34.close()
        pMain.close()

        # ---- overlapped with the ReduceScatter: FF chunk-0 weight loads ----
        wf = octx.enter_context(tc.tile_pool(name="wf", bufs=2))
        wf2 = octx.enter_context(tc.tile_pool(name="wf2", bufs=2))
        bfp = octx.enter_context(tc.tile_pool(name="bfp", bufs=1))

        def load_ff_chunk(c):
            a0, g0 = c * FCH, FFI + c * FCH
            w1a = [wf.tile([128, FCH], BF16, tag=f"w1a{k}", name=f"w1a{k}")
                   for k in range(8)]
            w1g = [wf.tile([128, FCH], BF16, tag=f"w1g{k}", name=f"w1g{k}")
                   for k in range(8)]
            for k in range(8):
                nc.sync.dma_start(
                    out=w1a[k][:],
                    in_=w_ff1_d[k * 128:(k + 1) * 128, a0:a0 + FCH])
                nc.sync.dma_start(
                    out=w1g[k][:],
                    in_=w_ff1_d[k * 128:(k + 1) * 128, g0:g0 + FCH])
            w2 = [wf2.tile([128, DM], BF16, tag=f"w2_{k}", name=f"w2_{k}")
                  for k in range(8)]
            for k in range(8):
                nc.sync.dma_start(
                    out=w2[k][:],
                    in_=w_ff2_d[c * FCH + k * 128:c * FCH + (k + 1) * 128, :])
            ba = [bfp.tile([128, 1], FP32, tag=f"ba{c}_{j}", name=f"ba{c}_{j}")
                  for j in range(8)]
            bg = [bfp.tile([128, 1], FP32, tag=f"bg{c}_{j}", name=f"bg{c}_{j}")
                  for j in range(8)]
            for j in range(8):
                nc.sync.dma_start(
                    out=ba[j][:],
                    in_=b_ff1_d[a0 + j * 128:a0 + (j + 1) * 128, :])
                nc.sync.dma_start(
                    out=bg[j][:],
                    in_=b_ff1_d[g0 + j * 128:g0 + (j + 1) * 128, :])
            return w1a, w1g, w2, ba, bg

        ff_chunk = load_ff_chunk(0)

        # ================= Phase 5: residual + LN2 (own L/4 slice) ==========
        h2fm_pool = octx.enter_context(tc.tile_pool(name="h2fm", bufs=8))
        x2p = octx.enter_context(tc.tile_pool(name="x2", bufs=1))
        xres_p = octx.enter_context(tc.tile_pool(name="xres", bufs=1))
        xres_sb = []
        for i in range(n_res):
            t = xres_p.tile([128, DM], FP32, tag=f"xres{i}", name=f"xres{i}")
            nc.sync.dma_start(out=t[:], in_=x_res_d[i * 128:(i + 1) * 128, :])
            xres_sb.append(t)
        x2_list = []
        with ExitStack() as p5, nc.named_scope("p5_ln2"):
            ld = p5.enter_context(tc.tile_pool(name="ld5", bufs=3))
            for i in range(n_res):
                r = slice(i * 128, (i + 1) * 128)
                mt = ld.tile([128, DM], BF16, tag="mr")
                nc.sync.dma_start(out=mt[:], in_=m_rs[r, :])
                x2 = x2p.tile([128, DM], FP32, tag=f"x2_{i}")
                nc.vector.tensor_add(x2[:], xres_sb[i][:], mt[:])
                x2_list.append(x2)
            h2fm = _layer_norm_stage(nc, tc, p5, x2_list, n_res, ident_sb,
                                     ln2_g, ln2_b, h2fm_pool, LS, "ln2")

        # ================= Phase 6: FF (full width, 4 chunks) ===============
        with ExitStack() as p6, nc.named_scope("p6_ff"):
            mma6 = p6.enter_context(tc.tile_pool(name="mma6", bufs=2,
                                                 space="PSUM"))
            mm6 = p6.enter_context(tc.tile_pool(name="mm6", bufs=2,
                                                space="PSUM"))
            pso_pool = p6.enter_context(tc.tile_pool(name="pso", bufs=2,
                                                     space="PSUM"))
            agp = p6.enter_context(tc.tile_pool(name="ag", bufs=2))
            tmp6 = p6.enter_context(tc.tile_pool(name="tmp6", bufs=4))
            acc_p = p6.enter_context(tc.tile_pool(name="ffacc", bufs=1))
            out_acc = [acc_p.tile([128, DM], FP32, tag=f"oacc{i}", name=f"oacc{i}")
                       for i in range(n_res)]

            for c in range(NCHK):
                w1a, w1g, w2, ba, bg = ff_chunk
                if c + 1 < NCHK:
                    next_chunk = load_ff_chunk(c + 1)
                ag_sb = []
                for sub in range(FCH // 128):
                    psA = mma6.tile([128, LS], FP32, tag="mma6")
                    psG = mm6.tile([128, LS], FP32, tag="mm6")
                    for k in range(8):
                        nc.tensor.matmul(
                            psA[:], w1a[k][:, sub * 128:(sub + 1) * 128],
                            h2fm[k][:], start=(k == 0), stop=(k == 7))
                    for k in range(8):
                        nc.tensor.matmul(
                            psG[:], w1g[k][:, sub * 128:(sub + 1) * 128],
                            h2fm[k][:], start=(k == 0), stop=(k == 7))
                    aa = tmp6.tile([128, LS], BF16, tag="aa")
                    nc.scalar.activation(aa[:], psA[:], ACTF.Identity,
                                         bias=ba[sub][:])
                    gg = tmp6.tile([128, LS], BF16, tag="gg")
                    nc.scalar.activation(gg[:], psG[:], ACTF.Gelu_apprx_tanh,
                                         bias=bg[sub][:])
                    agt = agp.tile([128, LS], BF16, tag=f"ag{sub}")
                    nc.vector.tensor_mul(agt[:], aa[:], gg[:])
                    ag_sb.append(agt)
                # ff2: token-major, accumulate chunks in SBUF fp32
                for tt in range(n_res):
                    pso = pso_pool.tile([128, DM], FP32, tag="pso")
                    for q in range(DM // 512):
                        for k in range(8):
                            nc.tensor.matmul(
                                pso[:, q * 512:(q + 1) * 512],
                                ag_sb[k][:, tt * 128:(tt + 1) * 128],
                                w2[k][:, q * 512:(q + 1) * 512],
                                start=(k == 0), stop=(k == 7))
                    if c == 0:
                        nc.any.tensor_copy(out_acc[tt][:], pso[:])
                    else:
                        nc.vector.tensor_add(out_acc[tt][:], out_acc[tt][:],
                                             pso[:])
                if c + 1 < NCHK:
                    ff_chunk = next_chunk

            # final: out = x2 + ff
            outp = p6.enter_context(tc.tile_pool(name="outp", bufs=2))
            for i in range(n_res):
                ot = outp.tile([128, DM], FP32, tag="ot")
                nc.vector.tensor_add(ot[:], x2_list[i][:], out_acc[i][:])
                nc.sync.dma_start(out=out_d[i * 128:(i + 1) * 128, :],
                                  in_=ot[:])
        nc.leave_named_scope("p4_outproj", _sid4, False)
    nc.compile()
    return nc


_NC_CACHE = {}


def _get_nc(L=L_FULL):
    if L not in _NC_CACHE:
        _NC_CACHE[L] = build_nc(L)
    return _NC_CACHE[L]


def make_in_maps(x, ln1_g, ln1_b, w_in, conv_w, conv_b, w_x, w_dt, b_dt,
                 a_log, d_skip, w_out, ln2_g, ln2_b, w_ff1, b_ff1, w_ff2,
                 b_ff2):
    x = np.asarray(x, np.float32)
    f32 = lambda a: np.ascontiguousarray(np.asarray(a, np.float32))
    bf = lambda a: np.ascontiguousarray(np.asarray(a, np.float32)).astype(NPBF16)
    ident = np.eye(128, dtype=np.float32).astype(NPBF16)
    a_neg = -np.exp(np.asarray(a_log, np.float32))
    w_ff1_b = bf(w_ff1)
    w_ff2_b = bf(w_ff2)
    b_ff1_c = f32(b_ff1).reshape(2 * FFI, 1)
    in_maps = []
    for c in range(8):
        b, s = c // 4, c % 4
        ds = slice(s * DIS, (s + 1) * DIS)
        in_maps.append(dict(
            x=bf(x[b]),
            x_res=f32(x[b][s * LS:(s + 1) * LS]),
            ln1_g=f32(ln1_g).reshape(DM, 1), ln1_b=f32(ln1_b).reshape(DM, 1),
            ln2_g=f32(ln2_g).reshape(DM, 1), ln2_b=f32(ln2_b).reshape(DM, 1),
            w_in=bf(np.concatenate(
                [w_in[:, s * DIS:(s + 1) * DIS],
                 w_in[:, DI + s * DIS:DI + (s + 1) * DIS]], axis=1)),
            conv_w=f32(conv_w[ds]), conv_b=f32(conv_b[ds]).reshape(DIS, 1),
            a_neg=f32(a_neg[ds]),
            w_x=bf(w_x[ds]), w_dt=bf(w_dt[:, ds]),
            b_dt=f32(b_dt[ds]).reshape(DIS, 1),
            d_skip=f32(d_skip[ds]).reshape(DIS, 1),
            w_out=bf(w_out[ds]),
            w_ff1=w_ff1_b, b_ff1=b_ff1_c, w_ff2=w_ff2_b,
            ident=ident,
        ))
    return in_maps


def combine_outputs(results, b_ff2, L=L_FULL):
    out = np.zeros((B_FULL, L, DM), np.float32)
    bff2 = np.asarray(b_ff2, np.float32)
    for b in range(B_FULL):
        for s in range(4):
            out[b, s * LS:(s + 1) * LS] = (
                results[4 * b + s]["out"].astype(np.float32) + bff2[None, :])
    return out


def kernel(**inputs):
    nc = _get_nc(L_FULL)
    in_maps = make_in_maps(
        inputs["x"], inputs["ln1_g"], inputs["ln1_b"], inputs["w_in"],
        inputs["conv_w"], inputs["conv_b"], inputs["w_x"], inputs["w_dt"],
        inputs["b_dt"], inputs["a_log"], inputs["d_skip"], inputs["w_out"],
        inputs["ln2_g"], inputs["ln2_b"], inputs["w_ff1"], inputs["b_ff1"],
        inputs["w_ff2"], inputs["b_ff2"])
    res = run_bass_kernel_spmd(nc, in_maps, core_ids=list(range(8)))
    return combine_outputs(res.results, inputs["b_ff2"], L_FULL)


# revision 57
# speedup vs baseline: 1.0522x; 1.0522x over previous
"""Trainium2 Bass kernel for BasicMambaBlock (B=2, L=2048, d_model=1024).

Sharding: 8 cores = 2 batch groups x 4 TP shards.
Mamba half: d_inner TP-4 (512 ch/core), feature-major activations,
scan via tensor_tensor_scan in two L-halves with chained state; the
y = sum_n C_n*h_n accumulation runs on the PE (identity-matmul into PSUM).
out_proj partials ReduceScatter'd so each core owns an L/4 token slice.
FF half: token-sliced (512 tok/core) with FULL GEGLU weights streamed in
4 column chunks; residual added on-chip, single (512,1024) fp32 output.
All activations except the FF gelu are composed from Exp/Ln only (silu via
sigmoid(v) = exp(-ln(1+exp(-v)))) and the act-table chooser is steered to
the combined natural_log_exp set, so the scalar engine loads an activation
table ~3 times total instead of ~54.
"""
import sys

sys.path.insert(0, "/opt/trn_rl_repo")

import numpy as np
import ml_dtypes
from contextlib import ExitStack

import concourse.bass as bass
import concourse.tile as tile
from concourse import bacc, mybir
from concourse.bass_utils import run_bass_kernel_spmd

FP32 = mybir.dt.float32
BF16 = mybir.dt.bfloat16
ALU = mybir.AluOpType
ACTF = mybir.ActivationFunctionType
NPBF16 = ml_dtypes.bfloat16

import os

# Timing-only ablation: replace the two collectives with local DMA copies
# (results become WRONG; used to isolate HW collective cost).
NOCOLL = bool(int(os.environ.get("KB_NOCOLL", "0")))

DM = 1024          # d_model
DI = 2048          # d_inner (global)
DIS = DI // 4      # 512 per-core d_inner shard
NST = 16           # d_state
RNK = 64           # dt_rank
DC = 4             # conv width
FFI = 4096         # ff inner (global)
EPS = 1e-5
L_FULL = 2048
B_FULL = 2
LS = L_FULL // 4   # 512-token output slice per core


def _silu_div(nc, scr, out, x_ap, d_scr_tag, bias=None, nbias=None):
    """out = silu(x + b) = (x + b) * sigmoid(x + b), with
    sigmoid(v) = exp(-softplus(-v)) = exp(-ln(1 + exp(-v))).

    3 Act ops (Exp/Ln/Exp, all in the natural_log_exp table set used by the
    rest of the kernel) plus 1 DVE op. bias/nbias are (128,1) APs for +b
    and -b (None -> 0).
    """
    shape = [x_ap.shape[0], x_ap.free_size()]
    e = scr.tile(shape, BF16, tag=d_scr_tag + "_e")
    nc.scalar.activation(e[:], x_ap, ACTF.Exp, scale=-1.0,
                         bias=(nbias if nbias is not None else 0.0))
    l = scr.tile(shape, BF16, tag=d_scr_tag + "_l")
    nc.scalar.activation(l[:], e[:], ACTF.Ln, bias=1.0)
    s = scr.tile(shape, BF16, tag=d_scr_tag + "_s")
    nc.scalar.activation(s[:], l[:], ACTF.Exp, scale=-1.0)
    if bias is None:
        nc.vector.scalar_tensor_tensor(out, x_ap, 1.0, s[:],
                                       ALU.mult, ALU.mult)
    else:
        nc.vector.scalar_tensor_tensor(out, x_ap, bias, s[:],
                                       ALU.add, ALU.mult)


def _layer_norm_stage(nc, tc, ctx, src_tiles, n_tok_tiles, ident_sb, g_ap, b_ap,
                      hfm_pool, L, name):
    """Token-major LN on src_tiles (list of (128, DM) fp32 sbuf tiles) ->
    feature-major bf16 tiles (8 x (128, L)), with g/b applied per-partition
    after the transpose. Returns list of 8 hfm tiles."""
    stat = ctx.enter_context(tc.tile_pool(name=f"{name}_stat", bufs=4))
    scr = ctx.enter_context(tc.tile_pool(name=f"{name}_scr", bufs=2))
    nrm = ctx.enter_context(tc.tile_pool(name=f"{name}_nrm", bufs=6))
    gsb = ctx.enter_context(tc.tile_pool(name=f"{name}_gb", bufs=1))

    # g/b per-feature: 8 x (128,1) tiles
    g_t, b_t = [], []
    for f in range(DM // 128):
        t = gsb.tile([128, 1], FP32, tag=f"g{f}")
        nc.sync.dma_start(out=t[:], in_=g_ap[f * 128:(f + 1) * 128, :])
        g_t.append(t)
        t = gsb.tile([128, 1], FP32, tag=f"b{f}")
        nc.sync.dma_start(out=t[:], in_=b_ap[f * 128:(f + 1) * 128, :])
        b_t.append(t)

    eps_t = gsb.tile([128, 1], FP32, tag="eps")
    nc.vector.memset(eps_t[:], EPS)

    # Group-of-4 processing: stats packed into columns, batched mu/rstd
    # math on (128,4) slices, then normalize + transpose the group so the
    # first feature-major chunks are ready before the full LN finishes.
    nt_ = n_tok_tiles
    TCH = 4  # token tiles per group / transpose chunk
    s1p = stat.tile([128, nt_], FP32, tag="s1p")
    s2p = stat.tile([128, nt_], FP32, tag="s2p")
    mu = stat.tile([128, nt_], FP32, tag="mu")
    musq = stat.tile([128, nt_], FP32, tag="musq")
    var = stat.tile([128, nt_], FP32, tag="var")
    lv = stat.tile([128, nt_], FP32, tag="lv")
    rstd = stat.tile([128, nt_], FP32, tag="rstd")
    psT = ctx.enter_context(tc.tile_pool(name=f"{name}_psT", bufs=2,
                                         space="PSUM"))
    hfm = []
    for f in range(DM // 128):
        ht = hfm_pool.tile([128, L], BF16, tag="hfm")
        hfm.append(ht)

    for c in range(n_tok_tiles // TCH):
        g0 = c * TCH
        gs = slice(g0, g0 + TCH)
        for i in range(g0, g0 + TCH):
            xt = src_tiles[i]
            nc.vector.tensor_reduce(s1p[:, i:i + 1], xt[:],
                                    mybir.AxisListType.X, ALU.add)
            sq = scr.tile([128, DM], FP32, tag="sq")
            nc.scalar.activation(sq[:], xt[:], ACTF.Square,
                                 accum_out=s2p[:, i:i + 1])
        nc.vector.tensor_scalar_mul(mu[:, gs], s1p[:, gs], 1.0 / DM)
        nc.vector.tensor_mul(musq[:, gs], mu[:, gs], mu[:, gs])
        nc.vector.scalar_tensor_tensor(var[:, gs], s2p[:, gs], 1.0 / DM,
                                       musq[:, gs], ALU.mult, ALU.subtract)
        nc.scalar.activation(lv[:, gs], var[:, gs], ACTF.Ln, bias=eps_t[:])
        nc.scalar.activation(rstd[:, gs], lv[:, gs], ACTF.Exp, scale=-0.5)
        normed = []
        for i in range(g0, g0 + TCH):
            nt = nrm.tile([128, DM], BF16, tag="normed")
            nc.vector.tensor_scalar(nt[:], src_tiles[i][:], mu[:, i:i + 1],
                                    rstd[:, i:i + 1], ALU.subtract, ALU.mult)
            normed.append(nt)
        for f in range(DM // 128):
            pt = psT.tile([128, TCH * 128], BF16, tag="psT")
            for i in range(TCH):
                nc.tensor.transpose(pt[:, i * 128:(i + 1) * 128],
                                    normed[i][:, f * 128:(f + 1) * 128],
                                    ident_sb[:])
            nc.any.tensor_scalar(
                hfm[f][:, g0 * 128:(g0 + TCH) * 128], pt[:],
                g_t[f][:], b_t[f][:], ALU.mult, ALU.add)
    return hfm


def build_nc(L=L_FULL):
    n_tok = L // 128
    CH = min(512, L)
    n_ch = L // CH  # token chunks for matmul moving dim
    LH = L // 2     # scan half length
    n_res = LS // 128  # token tiles in this core's residual/FF slice

    nc = bacc.Bacc("TRN2", target_bir_lowering=False, debug=False,
                   num_devices=8)

    # The act-table chooser is first-match per function, so an Exp/Ln mix
    # alternates between the exp-only and ln-only sets (a ~1.3us table load
    # per switch). Steer both functions to the combined natural_log_exp set
    # (a real act_info set containing exp+ln) by hiding them in the others.
    from concourse.hw_specs import get_activation_tables
    try:
        tabs = get_activation_tables(nc.m.arch)
        for tname, tset in tabs.items():
            if tname != "natural_log_exp_and_others":
                tset.discard(ACTF.Exp)
                tset.discard(ACTF.Ln)
    except Exception:
        pass

    # ---- dram params ----
    def din(name, shape, dt=FP32):
        return nc.dram_tensor(name, shape, dt, kind="ExternalInput").ap()

    x_d = din("x", [L, DM], BF16)
    x_res_d = din("x_res", [LS, DM])               # this core's token slice
    ln1_g = din("ln1_g", [DM, 1]); ln1_b = din("ln1_b", [DM, 1])
    ln2_g = din("ln2_g", [DM, 1]); ln2_b = din("ln2_b", [DM, 1])
    w_in_d = din("w_in", [DM, 2 * DIS], BF16)      # [xc cols | z cols]
    conv_w_d = din("conv_w", [DIS, DC])
    conv_b_d = din("conv_b", [DIS, 1])
    a_neg_d = din("a_neg", [DIS, NST])             # A = -exp(a_log) shard
    w_x_d = din("w_x", [DIS, RNK + 2 * NST], BF16)
    w_dt_d = din("w_dt", [RNK, DIS], BF16)
    b_dt_d = din("b_dt", [DIS, 1])
    d_skip_d = din("d_skip", [DIS, 1])
    w_out_d = din("w_out", [DIS, DM], BF16)
    w_ff1_d = din("w_ff1", [DM, 2 * FFI], BF16)    # FULL [a 4096 | g 4096]
    b_ff1_d = din("b_ff1", [2 * FFI, 1])           # FULL
    w_ff2_d = din("w_ff2", [FFI, DM], BF16)        # FULL
    ident_d = din("ident", [128, 128], BF16)

    out_d = nc.dram_tensor("out", [LS, DM], FP32, kind="ExternalOutput").ap()

    NCHK = 4               # FF column chunks
    FCH = FFI // NCHK      # 1024 a-cols + 1024 g-cols per chunk

    with tile.TileContext(nc) as tc, ExitStack() as octx:
        dram = octx.enter_context(tc.tile_pool(name="dram", bufs=1,
                                               space="DRAM"))
        const = octx.enter_context(tc.tile_pool(name="const", bufs=1))

        ident_sb = const.tile([128, 128], BF16, tag="ident")
        nc.sync.dma_start(out=ident_sb[:], in_=ident_d[:, :])

        # dram intermediates for collectives
        dbc_part = dram.tile([RNK + 2 * NST, L], BF16, tag="dbc_part")
        dbc_ar = dram.tile([RNK + 2 * NST, L], BF16, tag="dbc_ar")
        m_part = dram.tile([L, DM], BF16, tag="m_part")
        m_rs = dram.tile([LS, DM], BF16, tag="m_rs")

        groups = [[0, 1, 2, 3], [4, 5, 6, 7]]

        # Long-lived pools (phases 2-4) created first so shorter-lived
        # pools can pop in LIFO order before phase 5 reuses the space.
        pMain = octx.enter_context(ExitStack())
        wts = pMain.enter_context(tc.tile_pool(name="wts", bufs=1))
        sconst = pMain.enter_context(tc.tile_pool(name="sconst", bufs=1))
        act = pMain.enter_context(tc.tile_pool(name="act", bufs=1))
        zsp = pMain.enter_context(tc.tile_pool(name="zs", bufs=1))
        dtp = pMain.enter_context(tc.tile_pool(name="dtp", bufs=1))
        dtxp = pMain.enter_context(tc.tile_pool(name="dtx", bufs=1))
        dbcp = pMain.enter_context(tc.tile_pool(name="dbcp", bufs=1))

        # ================= Phase 1: LN1 -> h_fm =================
        p12 = pMain.enter_context(ExitStack())
        mm = p12.enter_context(tc.tile_pool(name="mm", bufs=4, space="PSUM"))
        hfm_pool = p12.enter_context(tc.tile_pool(name="hfm", bufs=8))
        with ExitStack() as p1, nc.named_scope("p1_ln1"):
            xload = p1.enter_context(tc.tile_pool(name="xload", bufs=6))
            xt_list = []
            for i in range(n_tok):
                xt = xload.tile([128, DM], BF16, tag="xt")
                nc.sync.dma_start(out=xt[:], in_=x_d[i * 128:(i + 1) * 128, :])
                xt_list.append(xt)
            hfm = _layer_norm_stage(nc, tc, p1, xt_list, n_tok, ident_sb,
                                    ln1_g, ln1_b, hfm_pool, L, "ln1")

        # ================= Phase 2: in_proj, conv, dbc, dt =================
        _sid2 = nc.enter_named_scope("p2_inproj", False)[0]
        w12 = p12.enter_context(tc.tile_pool(name="w12", bufs=1))
        w_in_sb = []
        for k in range(8):
            t = w12.tile([128, 2 * DIS], BF16, tag=f"w_in{k}")
            nc.sync.dma_start(out=t[:], in_=w_in_d[k * 128:(k + 1) * 128, :])
            w_in_sb.append(t)
        wx_sb = []
        for k in range(4):
            t = w12.tile([128, RNK + 2 * NST], BF16, tag=f"wx{k}")
            nc.sync.dma_start(out=t[:], in_=w_x_d[k * 128:(k + 1) * 128, :])
            wx_sb.append(t)
        wdt_sb = w12.tile([RNK, DIS], BF16, tag="wdt")
        nc.sync.dma_start(out=wdt_sb[:], in_=w_dt_d[:, :])

        cw_sb, cb_sb, cbn_sb, a_sb, bdt_sb, dskip_sb = [], [], [], [], [], []
        for d in range(4):
            r = slice(d * 128, (d + 1) * 128)
            t = sconst.tile([128, DC], FP32, tag=f"cw{d}")
            nc.sync.dma_start(out=t[:], in_=conv_w_d[r, :]); cw_sb.append(t)
            t = sconst.tile([128, 1], FP32, tag=f"cb{d}")
            nc.sync.dma_start(out=t[:], in_=conv_b_d[r, :]); cb_sb.append(t)
            tn = sconst.tile([128, 1], FP32, tag=f"cbn{d}")
            nc.vector.tensor_scalar_mul(tn[:], t[:], -1.0); cbn_sb.append(tn)
            t = sconst.tile([128, NST], FP32, tag=f"a{d}")
            nc.sync.dma_start(out=t[:], in_=a_neg_d[r, :]); a_sb.append(t)
            t = sconst.tile([128, 1], FP32, tag=f"bdt{d}")
            nc.sync.dma_start(out=t[:], in_=b_dt_d[r, :]); bdt_sb.append(t)
            t = sconst.tile([128, 1], FP32, tag=f"dsk{d}")
            nc.sync.dma_start(out=t[:], in_=d_skip_d[r, :]); dskip_sb.append(t)

        xc_pad = []
        for d in range(4):
            t = act.tile([128, L + 3], BF16, tag=f"xcp{d}")
            nc.vector.memset(t[:, 0:3], 0.0)
            xc_pad.append(t)
        zraw = p12.enter_context(tc.tile_pool(name="zraw", bufs=1))
        z_sb = [zraw.tile([128, L], BF16, tag=f"z{d}", name=f"z{d}")
                for d in range(4)]

        # in_proj xc columns only (z columns are deferred into the
        # dbc-AllReduce window below)
        def inproj_f(f):
            for c in range(n_ch):
                ps = mm.tile([128, CH], FP32, tag="mm", name="ps")
                for k in range(8):
                    nc.tensor.matmul(
                        ps[:], w_in_sb[k][:, f * 128:(f + 1) * 128],
                        hfm[k][:, c * CH:(c + 1) * CH],
                        start=(k == 0), stop=(k == 7))
                if f < 4:
                    nc.any.tensor_copy(
                        xc_pad[f][:, 3 + c * CH: 3 + (c + 1) * CH], ps[:])
                else:
                    nc.any.tensor_copy(
                        z_sb[f - 4][:, c * CH:(c + 1) * CH], ps[:])

        for f in range(4):
            inproj_f(f)

        # conv + silu (writes silu'd xc back into xc_pad[:, 3:3+L])
        cacc = p12.enter_context(tc.tile_pool(name="cacc", bufs=2))
        for d in range(4):
            acc = cacc.tile([128, L], BF16, tag="cacc")
            nc.vector.tensor_scalar_mul(acc[:], xc_pad[d][:, 0:L],
                                        cw_sb[d][:, 0:1])
            for j in range(1, DC):
                nc.vector.scalar_tensor_tensor(
                    acc[:], xc_pad[d][:, j:j + L], cw_sb[d][:, j:j + 1],
                    acc[:], ALU.mult, ALU.add)
            _silu_div(nc, cacc, xc_pad[d][:, 3:3 + L], acc[:], "sil",
                      bias=cb_sb[d][:], nbias=cbn_sb[d][:])

        # dbc partial + AllReduce
        nc.leave_named_scope("p2_inproj", _sid2, False)
        _sid2b = nc.enter_named_scope("p2b_dbc_ar", False)[0]
        dbc_sb = dbcp.tile([RNK + 2 * NST, L], BF16, tag="dbc")
        for c in range(n_ch):
            ps = mm.tile([RNK + 2 * NST, CH], FP32, tag="mm")
            for k in range(4):
                nc.tensor.matmul(ps[:], wx_sb[k][:],
                                 xc_pad[k][:, 3 + c * CH:3 + (c + 1) * CH],
                                 start=(k == 0), stop=(k == 3))
            nc.any.tensor_copy(dbc_sb[:, c * CH:(c + 1) * CH], ps[:])
        nc.gpsimd.dma_start(out=dbc_part[:], in_=dbc_sb[:])
        if NOCOLL:
            nc.sync.dma_start(out=dbc_ar[:], in_=dbc_part[:])
        else:
            nc.gpsimd.collective_compute(
                "AllReduce", ALU.add, replica_groups=groups,
                ins=[dbc_part.opt()], outs=[dbc_ar.opt()])

        # ---- overlapped with the AllReduce: z in_proj, z-silu, prefetches
        for f in range(4, 8):
            inproj_f(f)
        zs_sb = []
        for d in range(4):
            zs = zsp.tile([128, L], BF16, tag=f"zs{d}")
            _silu_div(nc, cacc, zs[:], z_sb[d][:], "sil")
            zs_sb.append(zs)
        wout_sb = []
        for k in range(4):
            t = wts.tile([128, DM], BF16, tag=f"wout{k}")
            nc.sync.dma_start(out=t[:], in_=w_out_d[k * 128:(k + 1) * 128, :])
            wout_sb.append(t)

        nc.sync.dma_start(out=dbc_sb[:], in_=dbc_ar[:])
        dtlo = dbc_sb[0:RNK, :]
        nc.leave_named_scope("p2b_dbc_ar", _sid2b, False)
        _sid2c = nc.enter_named_scope("p2c_dt", False)[0]

        # dt = softplus(dt_lo @ w_dt + b_dt) = Ln(Exp(u + b_dt) + 1)
        # chunk-outer so the scan's half-0 inputs (dt/dtx chunks 0,1 of
        # every d) are ready as early as possible
        spscr = p12.enter_context(tc.tile_pool(name="spscr", bufs=3))
        dt_sb = [dtp.tile([128, L], BF16, tag=f"dt{d}", name=f"dt{d}")
                 for d in range(4)]
        dtx_sb = [dtxp.tile([128, L], BF16, tag=f"dtx{d}", name=f"dtx{d}")
                  for d in range(4)]
        for c in range(n_ch):
            cs = slice(c * CH, (c + 1) * CH)
            for d in range(4):
                ps = mm.tile([128, CH], FP32, tag="mm", name="ps")
                nc.tensor.matmul(ps[:], wdt_sb[:, d * 128:(d + 1) * 128],
                                 dtlo[:, cs].opt(),
                                 start=True, stop=True)
                e = spscr.tile([128, CH], FP32, tag="sp_e")
                nc.scalar.activation(e[:], ps[:], ACTF.Exp, bias=bdt_sb[d][:])
                nc.scalar.activation(dt_sb[d][:, cs], e[:], ACTF.Ln, bias=1.0)
                nc.vector.tensor_mul(dtx_sb[d][:, cs], dt_sb[d][:, cs],
                                     xc_pad[d][:, 3 + c * CH:3 + (c + 1) * CH])
        p12.close()

        nc.leave_named_scope("p2c_dt", _sid2c, False)
        _sid3 = nc.enter_named_scope("p3_scan", False)[0]

        # ================= Phase 3: scan + gate =================
        # Two L-halves; y_d accumulated in PSUM via identity matmul over n.
        p34 = pMain.enter_context(ExitStack())
        gp = p34.enter_context(tc.tile_pool(name="gated", bufs=1))
        p3 = p34.enter_context(ExitStack())
        sc = p3.enter_context(tc.tile_pool(name="scan", bufs=4))
        bcp = p3.enter_context(tc.tile_pool(name="bcast", bufs=5))
        ypsum = p3.enter_context(tc.tile_pool(name="ypsum", bufs=1,
                                              space="PSUM"))
        hlast = [sconst.tile([128, NST], FP32, tag=f"hl{d}", name=f"hl{d}")
                 for d in range(4)]
        gated_sb = [gp.tile([128, L], BF16, tag=f"g{d}", name=f"g{d}")
                    for d in range(4)]

        for h in range(2):
            off = h * LH
            y_ps = [ypsum.tile([128, LH], FP32, tag=f"yps{d}", name=f"yps{d}")
                    for d in range(4)]
            for n in range(NST):
                bcB = bcp.tile([128, LH], BF16, tag="bcB")
                nc.sync.dma_start(
                    out=bcB[:],
                    in_=dbc_ar[RNK + n:RNK + n + 1,
                               off:off + LH].broadcast_to([128, LH]))
                bcC = bcp.tile([128, LH], BF16, tag="bcC")
                nc.sync.dma_start(
                    out=bcC[:],
                    in_=dbc_ar[RNK + NST + n:RNK + NST + n + 1,
                               off:off + LH].broadcast_to([128, LH]))
                for d in range(4):
                    dA = sc.tile([128, LH], BF16, tag="dA")
                    nc.scalar.activation(dA[:], dt_sb[d][:, off:off + LH],
                                         ACTF.Exp, scale=a_sb[d][:, n:n + 1])
                    dB = sc.tile([128, LH], BF16, tag="dB")
                    nc.vector.tensor_mul(dB[:], dtx_sb[d][:, off:off + LH],
                                         bcB[:])
                    # h state scan (in-place over dB), fp32 internal state
                    init = 0.0 if h == 0 else hlast[d][:, n:n + 1]
                    nc.vector.tensor_tensor_scan(dB[:], dA[:], dB[:], init,
                                                 ALU.mult, ALU.add)
                    if h == 0:
                        nc.scalar.activation(hlast[d][:, n:n + 1],
                                             dB[:, LH - 1:LH], ACTF.Identity)
                    # hC into dA tile (reuse), then accumulate on PE
                    # (matmul output is capped at one PSUM bank = 512 fp32).
                    # Half the hC muls go to the otherwise-idle Pool engine
                    # to offload the DVE (the scan-phase bottleneck).
                    if d < 3:
                        nc.gpsimd.tensor_mul(dA[:], dB[:], bcC[:])
                    else:
                        nc.vector.tensor_mul(dA[:], dB[:], bcC[:])
                    for q in range(LH // 512):
                        nc.tensor.matmul(
                            y_ps[d][:, q * 512:(q + 1) * 512], ident_sb[:],
                            dA[:, q * 512:(q + 1) * 512],
                            start=(n == 0), stop=(n == NST - 1))
            # gate: gated = (y + d_skip*xc) * silu(z)
            for d in range(4):
                tmp = sc.tile([128, LH], FP32, tag="gtmp")
                nc.vector.scalar_tensor_tensor(
                    tmp[:], xc_pad[d][:, 3 + off:3 + off + LH],
                    dskip_sb[d][:], y_ps[d][:], ALU.mult, ALU.add)
                nc.gpsimd.tensor_mul(gated_sb[d][:, off:off + LH], tmp[:],
                                      zs_sb[d][:, off:off + LH])

        nc.leave_named_scope("p3_scan", _sid3, False)
        _sid4 = nc.enter_named_scope("p4_outproj", False)[0]

        # ================= Phase 4: out_proj + ReduceScatter =================
        p3.close()
        mmo = p34.enter_context(tc.tile_pool(name="mmo", bufs=2, space="PSUM"))
        mm2 = p34.enter_context(tc.tile_pool(name="mm2", bufs=2, space="PSUM"))
        mp_pool = p34.enter_context(tc.tile_pool(name="mp", bufs=3))
        for i in range(n_tok):
            mp = mp_pool.tile([128, DM], BF16, tag="mp")
            for nchk in range(2):
                pool = mmo if nchk == 0 else mm2
                ps = pool.tile([128, 512], FP32, tag=pool.name)
                for k in range(4):
                    nc.tensor.matmul(
                        ps[:], gated_sb[k][:, i * 128:(i + 1) * 128],
                        wout_sb[k][:, nchk * 512:(nchk + 1) * 512],
                        start=(k == 0), stop=(k == 3))
                nc.any.tensor_copy(mp[:, nchk * 512:(nchk + 1) * 512], ps[:])
            nc.sync.dma_start(out=m_part[i * 128:(i + 1) * 128, :], in_=mp[:])
        if NOCOLL:
            nc.sync.dma_start(out=m_rs[:], in_=m_part[0:LS, :])
        else:
            nc.gpsimd.collective_compute(
                "ReduceScatter", ALU.add, replica_groups=groups,
                ins=[m_part.opt()], outs=[m_rs.opt()])
        p34.close()
        pMain.close()

        # ---- overlapped with the ReduceScatter: FF chunk-0 weight loads ----
        wf = octx.enter_context(tc.tile_pool(name="wf", bufs=2))
        wf2 = octx.enter_context(tc.tile_pool(name="wf2", bufs=2))
        bfp = octx.enter_context(tc.tile_pool(name="bfp", bufs=1))

        def load_ff_chunk(c):
            a0, g0 = c * FCH, FFI + c * FCH
            w1a = [wf.tile([128, FCH], BF16, tag=f"w1a{k}", name=f"w1a{k}")
                   for k in range(8)]
            w1g = [wf.tile([128, FCH], BF16, tag=f"w1g{k}", name=f"w1g{k}")
                   for k in range(8)]
            for k in range(8):
                nc.sync.dma_start(
                    out=w1a[k][:],
                    in_=w_ff1_d[k * 128:(k + 1) * 128, a0:a0 + FCH])
                nc.sync.dma_start(
                    out=w1g[k][:],
                    in_=w_ff1_d[k * 128:(k + 1) * 128, g0:g0 + FCH])
            w2 = [wf2.tile([128, DM], BF16, tag=f"w2_{k}", name=f"w2_{k}")
                  for k in range(8)]
            for k in range(8):
                nc.sync.dma_start(
                    out=w2[k][:],
                    in_=w_ff2_d[c * FCH + k * 128:c * FCH + (k + 1) * 128, :])
            ba = [bfp.tile([128, 1], FP32, tag=f"ba{c}_{j}", name=f"ba{c}_{j}")
                  for j in range(8)]
            bg = [bfp.tile([128, 1], FP32, tag=f"bg{c}_{j}", name=f"bg{c}_{j}")
                  for j in range(8)]
            for j in range(8):
                nc.sync.dma_start(
                    out=ba[j][:],
                    in_=b_ff1_d[a0 + j * 128:a0 + (j + 1) * 128, :])
                nc.sync.dma_start(
                    out=bg[j][:],
                    in_=b_ff1_d[g0 + j * 128:g0 + (j + 1) * 128, :])
            return w1a, w1g, w2, ba, bg

        ff_chunk = load_ff_chunk(0)

        # ================= Phase 5: residual + LN2 (own L/4 slice) ==========
        h2fm_pool = octx.enter_context(tc.tile_pool(name="h2fm", bufs=8))
        x2p = octx.enter_context(tc.tile_pool(name="x2", bufs=1))
        xres_p = octx.enter_context(tc.tile_pool(name="xres", bufs=1))
        xres_sb = []
        for i in range(n_res):
            t = xres_p.tile([128, DM], FP32, tag=f"xres{i}", name=f"xres{i}")
            nc.sync.dma_start(out=t[:], in_=x_res_d[i * 128:(i + 1) * 128, :])
            xres_sb.append(t)
        x2_list = []
        with ExitStack() as p5, nc.named_scope("p5_ln2"):
            ld = p5.enter_context(tc.tile_pool(name="ld5", bufs=3))
            for i in range(n_res):
                r = slice(i * 128, (i + 1) * 128)
                mt = ld.tile([128, DM], BF16, tag="mr")
                nc.sync.dma_start(out=mt[:], in_=m_rs[r, :])
                x2 = x2p.tile([128, DM], FP32, tag=f"x2_{i}")
                nc.vector.tensor_add(x2[:], xres_sb[i][:], mt[:])
                x2_list.append(x2)
            h2fm = _layer_norm_stage(nc, tc, p5, x2_list, n_res, ident_sb,
                                     ln2_g, ln2_b, h2fm_pool, LS, "ln2")

        # ================= Phase 6: FF (full width, 4 chunks) ===============
        with ExitStack() as p6, nc.named_scope("p6_ff"):
            mma6 = p6.enter_context(tc.tile_pool(name="mma6", bufs=2,
                                                 space="PSUM"))
            mm6 = p6.enter_context(tc.tile_pool(name="mm6", bufs=2,
                                                space="PSUM"))
            pso_pool = p6.enter_context(tc.tile_pool(name="pso", bufs=2,
                                                     space="PSUM"))
            agp = p6.enter_context(tc.tile_pool(name="ag", bufs=2))
            tmp6 = p6.enter_context(tc.tile_pool(name="tmp6", bufs=4))
            acc_p = p6.enter_context(tc.tile_pool(name="ffacc", bufs=1))
            out_acc = [acc_p.tile([128, DM], FP32, tag=f"oacc{i}", name=f"oacc{i}")
                       for i in range(n_res)]

            for c in range(NCHK):
                w1a, w1g, w2, ba, bg = ff_chunk
                if c + 1 < NCHK:
                    next_chunk = load_ff_chunk(c + 1)
                ag_sb = []
                for sub in range(FCH // 128):
                    psA = mma6.tile([128, LS], FP32, tag="mma6")
                    psG = mm6.tile([128, LS], FP32, tag="mm6")
                    for k in range(8):
                        nc.tensor.matmul(
                            psA[:], w1a[k][:, sub * 128:(sub + 1) * 128],
                            h2fm[k][:], start=(k == 0), stop=(k == 7))
                    for k in range(8):
                        nc.tensor.matmul(
                            psG[:], w1g[k][:, sub * 128:(sub + 1) * 128],
                            h2fm[k][:], start=(k == 0), stop=(k == 7))
                    aa = tmp6.tile([128, LS], BF16, tag="aa")
                    nc.scalar.activation(aa[:], psA[:], ACTF.Identity,
                                         bias=ba[sub][:])
                    gg = tmp6.tile([128, LS], BF16, tag="gg")
                    nc.scalar.activation(gg[:], psG[:], ACTF.Gelu_apprx_tanh,
                                         bias=bg[sub][:])
                    agt = agp.tile([128, LS], BF16, tag=f"ag{sub}")
                    nc.vector.tensor_mul(agt[:], aa[:], gg[:])
                    ag_sb.append(agt)
                # ff2: token-major, accumulate chunks in SBUF fp32
                for tt in range(n_res):
                    pso = pso_pool.tile([128, DM], FP32, tag="pso")
                    for q in range(DM // 512):
                        for k in range(8):
                            nc.tensor.matmul(
                                pso[:, q * 512:(q + 1) * 512],
                                ag_sb[k][:, tt * 128:(tt + 1) * 128],
                                w2[k][:, q * 512:(q + 1) * 512],
                                start=(k == 0), stop=(k == 7))
                    if c == 0:
                        nc.any.tensor_copy(out_acc[tt][:], pso[:])
                    else:
                        nc.vector.tensor_add(out_acc[tt][:], out_acc[tt][:],
                                             pso[:])
                if c + 1 < NCHK:
                    ff_chunk = next_chunk

            # final: out = x2 + ff
            outp = p6.enter_context(tc.tile_pool(name="outp", bufs=2))
            for i in range(n_res):
                ot = outp.tile([128, DM], FP32, tag="ot")
                nc.vector.tensor_add(ot[:], x2_list[i][:], out_acc[i][:])
                nc.sync.dma_start(out=out_d[i * 128:(i + 1) * 128, :],
                                  in_=ot[:])
        nc.leave_named_scope("p4_outproj", _sid4, False)
    nc.compile()
    return nc


_NC_CACHE = {}


def _get_nc(L=L_FULL):
    if L not in _NC_CACHE:
        _NC_CACHE[L] = build_nc(L)
    return _NC_CACHE[L]


def make_in_maps(x, ln1_g, ln1_b, w_in, conv_w, conv_b, w_x, w_dt, b_dt,
                 a_log, d_skip, w_out, ln2_g, ln2_b, w_ff1, b_ff1, w_ff2,
                 b_ff2):
    x = np.asarray(x, np.float32)
    f32 = lambda a: np.ascontiguousarray(np.asarray(a, np.float32))
    bf = lambda a: np.ascontiguousarray(np.asarray(a, np.float32)).astype(NPBF16)
    ident = np.eye(128, dtype=np.float32).astype(NPBF16)
    a_neg = -np.exp(np.asarray(a_log, np.float32))
    w_ff1_b = bf(w_ff1)
    w_ff2_b = bf(w_ff2)
    b_ff1_c = f32(b_ff1).reshape(2 * FFI, 1)
    in_maps = []
    for c in range(8):
        b, s = c // 4, c % 4
        ds = slice(s * DIS, (s + 1) * DIS)
        in_maps.append(dict(
            x=bf(x[b]),
            x_res=f32(x[b][s * LS:(s + 1) * LS]),
            ln1_g=f32(ln1_g).reshape(DM, 1), ln1_b=f32(ln1_b).reshape(DM, 1),
            ln2_g=f32(ln2_g).reshape(DM, 1), ln2_b=f32(ln2_b).reshape(DM, 1),
            w_in=bf(np.concatenate(
                [w_in[:, s * DIS:(s + 1) * DIS],
                 w_in[:, DI + s * DIS:DI + (s + 1) * DIS]], axis=1)),
            conv_w=f32(conv_w[ds]), conv_b=f32(conv_b[ds]).reshape(DIS, 1),
            a_neg=f32(a_neg[ds]),
            w_x=bf(w_x[ds]), w_dt=bf(w_dt[:, ds]),
            b_dt=f32(b_dt[ds]).reshape(DIS, 1),
            d_skip=f32(d_skip[ds]).reshape(DIS, 1),
            w_out=bf(w_out[ds]),
            w_ff1=w_ff1_b, b_ff1=b_ff1_c, w_ff2=w_ff2_b,
            ident=ident,
        ))
    return in_maps


def combine_outputs(results, b_ff2, L=L_FULL):
    out = np.zeros((B_FULL, L, DM), np.float32)
    bff2 = np.asarray(b_ff2, np.float32)
    for b in range(B_FULL):
        for s in range(4):
            out[b, s * LS:(s + 1) * LS] = (
                results[4 * b + s]["out"].astype(np.float32) + bff2[None, :])
    return out


def kernel(**inputs):
    nc = _get_nc(L_FULL)
    in_maps = make_in_maps(
        inputs["x"], inputs["ln1_g"], inputs["ln1_b"], inputs["w_in"],
        inputs["conv_w"], inputs["conv_b"], inputs["w_x"], inputs["w_dt"],
        inputs["b_dt"], inputs["a_log"], inputs["d_skip"], inputs["w_out"],
        inputs["ln2_g"], inputs["ln2_b"], inputs["w_ff1"], inputs["b_ff1"],
        inputs["w_ff2"], inputs["b_ff2"])
    res = run_bass_kernel_spmd(nc, in_maps, core_ids=list(range(8)))
    return combine_outputs(res.results, inputs["b_ff2"], L_FULL)
